# revision 56
# baseline (speedup 1.0000x reference)
"""Multi-head attention (B=2, S=4096, D=512, H=8) on 8 Trainium2 NeuronCores.

Sharding: batch x head-pair parallelism. Core c handles batch b = c // 4 and
heads {2*(c%4), 2*(c%4)+1} (128 contiguous rows of the QKV projection
weights, Megatron column-parallel; Wo row-parallel with the partial-sum
reduction done on the host at gather time).

Per-core device program (matmul operands bf16, accumulation fp32 PSUM).
The PE array is the critical engine (~85% busy); the structure keeps its
matmul stream dense while ACT and DVE split the exp/normalize load:
  - Input DMA on both HWDGE queues (Sync + Activation) in parallel,
    need-ordered; the Activation queue carries only the small warmup set
    so its FIFO is clear before the first exp (queued DMA issues block on
    DMA-ring credits). K-block0/Q-block0/V-group0 projected up front;
    remaining blocks injected just-in-time into qb0's ktg loop.
  - Scores per key tile: ONE [128(k), 1024] f32 PSUM tile holds BOTH
    heads ([h0 512 | h1 512]), so the row-tiled K=64 pair (h0 partitions
    0-63, h1 64-127) shares its psum-free dependency, issues adjacently,
    and co-executes in the PE array (2x score throughput).
  - exp: part 0 of each ktg on ACT (exact exp, PSUM->SBUF bf16), part 1
    on DVE via a Schraudolph bit-hack (int16(x*128/ln2 + 16251) bitcast
    bf16, ~3% max rel err). Splitting each ktg across both engines halves
    exp latency per ktg, so the lag-1 AV drain at qblock boundaries is
    short, and the engines stay load-balanced.
  - AV: vh tiles [128, 65] (65th col = ones -> softmax denominator) as
    stationary; accumulated over all 32 key tiles directly in PSUM
    ([65, 512] per head, held across the qblock), one ktg behind scores.
  - Normalize (deferred into the next qblock): av+den evacuated in one
    bf16 cast per head (h0 on DVE, h1 on ACT), denominators moved to a
    q-on-partitions layout with tiny K=1 matmuls, ONE [128, 8] exact
    reciprocal, per-head output projection, 1/den applied per-partition
    at evacuation (h0 DVE tensor_scalar, h1 ACT activation-scale), and
    per-head bf16 partials DMA'd out separately (rows [0:S] = head0,
    [S:2S] = head1) -- the head sum rides the host gather.

Host gathers: out[b] = sum over 4 cores of (head0 + head1 partials)
+ bv @ Wo.T + bo.
"""

from collections import defaultdict

import ml_dtypes
import numpy as np

import concourse.mybir as mybir
import concourse.tile as tile
from concourse import bacc
from concourse.bass_utils import run_bass_kernel_spmd

F32 = mybir.dt.float32
BF16 = mybir.dt.bfloat16
I16 = mybir.dt.int16
EXP = mybir.ActivationFunctionType.Exp
ADD = mybir.AluOpType.add
MULT = mybir.AluOpType.mult
NPBF16 = ml_dtypes.bfloat16

B, S, D, H = 2, 4096, 512, 8
DK = D // H          # 64
HPC = 2              # heads per core
HD = HPC * DK        # 128 head-dims per core
N_CORES = 8
QB = 512             # query block (matmul free dim)
KT = 128             # key tile (partition dim)
NCH = D // 128       # 4 contraction chunks for the projections
KPG = 2              # key tiles per score/exp group
SCW = KPG * QB       # score tile width (1024)
NSUB = QB // 128     # out-projection subtiles per qblock (4)

# Schraudolph bf16 exp: exp(s) ~= bitcast_bf16(int16(s*C1 + C2)); C1 = 2^7/ln2,
# C2 = 127*128 - sigma with sigma tuned for truncating f32->int16 conversion.
EXP_C1 = 184.6649652337873
EXP_C2 = 16251.0
# exp engine assignment: part 0 of each ktg runs on ACT, part 1 on DVE for
# ktg >= DVE_PART_MIN (the first ktgs go all-ACT to balance DVE's fixed
# norm/copy load). Splitting each ktg across both engines halves the exp
# latency per ktg, so the lag-1 AV drain at the qblock boundary is short.
DVE_PART_MIN = 2


def mha_tile_kernel(tc, out_ap, ins, seq=S, dve_part_min=DVE_PART_MIN):
    nc = tc.nc
    nqb, nkt = seq // QB, seq // KT
    nktg = nkt // KPG
    nst = seq // 128                      # 128-wide s-subtiles for V
    vgroups = [range(0, min(4, nst)), range(4, min(8, nst))] + [
        range(g, min(g + 8, nst)) for g in range(8, nst, 8)
    ]
    vgroups = [g for g in vgroups if len(g) > 0]

    xq, xk, xv = ins["qt"], ins["kt"], ins["vt"]
    const = tc.alloc_tile_pool(name="const", bufs=1)
    sb = tc.alloc_tile_pool(name="sb", bufs=2)
    scp = tc.alloc_tile_pool(name="scp", bufs=3, space="PSUM")
    avp = tc.alloc_tile_pool(name="avp", bufs=2, space="PSUM")

    # --- weights / constants ---
    wq_sb = const.tile([128, NCH, 128], BF16, tag="wq", name="wq_sb")
    wk_sb = const.tile([128, NCH, 128], BF16, tag="wk", name="wk_sb")
    wv_sb = const.tile([128, NCH, 128], BF16, tag="wv", name="wv_sb")
    wo0_sb = const.tile([64, QB], BF16, tag="wo0", name="wo0_sb")
    wo1_sb = const.tile([64, QB], BF16, tag="wo1", name="wo1_sb")
    bq_sb = const.tile([128, 1], F32, tag="bq", name="bq_sb")
    bk_sb = const.tile([128, 1], F32, tag="bk", name="bk_sb")
    ones_sb = const.tile([128, 64], F32, tag="ones", name="ones_sb")
    nc.vector.memset(ones_sb, 1.0)
    ones_bf = const.tile([128, 1], BF16, tag="onesb", name="ones_bf")
    nc.vector.memset(ones_bf, 1.0)

    # --- persistent activations ---
    qhT = const.tile([128, seq], BF16, tag="qhT", name="qhT")
    khT = const.tile([128, seq], BF16, tag="khT", name="khT")
    vh = [
        const.tile([128, nkt * 65], BF16, tag=f"vh{h}", name=f"vh{h}")
        for h in range(HPC)
    ]
    for h in range(HPC):
        ones_col = vh[h].rearrange("p (j c) -> p j c", c=65)[:, :, 64]
        nc.vector.tensor_copy(out=ones_col, in_=ones_sb[:, 0:nkt])

    # --- raw inputs in SBUF; DMA'd in [128, 512] slices in consumption order
    xk_sb = [const.tile([128, seq], BF16, tag=f"xk{c}", name=f"xk{c}") for c in range(NCH)]
    xq_sb = [const.tile([128, seq], BF16, tag=f"xq{c}", name=f"xq{c}") for c in range(NCH)]
    xv_sb = [const.tile([128, seq], BF16, tag=f"xv{c}", name=f"xv{c}") for c in range(NCH)]

    def dma_x(dst_tiles, src, j, eng=None, width=1):
        sl = slice(j * QB, (j + width) * QB)
        for c in range(NCH):
            (eng or nc.sync).dma_start(
                dst_tiles[c][:, sl], src[c * 128 : (c + 1) * 128, sl]
            )

    # DMAs issue serially per queue (~640ns each after a ~6.6us preamble);
    # both the Sync and Activation queues are HWDGE-capable, so the warmup
    # transfers (K0 on scalar, Q0 on sync) issue in parallel. Weights are
    # host-permuted to [p, c, m] so their DMA is contiguous.
    # Two HWDGE queues (Sync + Activation) issue in parallel. The Scalar
    # queue gets ONLY the small warmup set (its FIFO must be clear before
    # the first exp ACTIVATE -- queued DMA issues block on DMA-ring credits
    # for tens of us). Everything else goes need-ordered on Sync.
    # the two transfers gating the FIRST matmul (wk + xk0 chunk0) lead the
    # two queues in parallel so they land with an empty DMA pipe
    nc.scalar.dma_start(wk_sb, ins["wk"].rearrange("p (c m) -> p c m", m=128))
    nc.sync.dma_start(xk_sb[0][:, 0:QB], xk[0:128, 0:QB])
    for c in range(1, NCH):
        nc.scalar.dma_start(xk_sb[c][:, 0:QB], xk[c * 128 : (c + 1) * 128, 0:QB])
    nc.scalar.dma_start(bk_sb, ins["bk"])
    nc.scalar.dma_start(wv_sb, ins["wv"].rearrange("p (c m) -> p c m", m=128))
    dma_x(xv_sb, xv, 0, eng=nc.scalar)            # v subtiles 0-3 @ qb0 ktg1
    nc.sync.dma_start(wq_sb, ins["wq"].rearrange("p (c m) -> p c m", m=128))
    dma_x(xq_sb, xq, 0, eng=nc.sync)
    nc.sync.dma_start(bq_sb, ins["bq"])
    if nqb > 1:
        dma_x(xk_sb, xk, 1, eng=nc.sync)          # K1-proj @ qb0 ktg0
        dma_x(xv_sb, xv, 1, eng=nc.sync)          # v subtiles 4-7 @ qb0 ktg1
    nc.sync.dma_start(wo0_sb, ins["wo"][0:64, :])  # first norm_b in qb1
    nc.sync.dma_start(wo1_sb, ins["wo"][64:128, :])
    # remaining blocks on Sync by first-use slot (K j @ ktg 2j-2, V grp g
    # @ 4g-1); 1024-wide slices to halve the issue count
    rest = []
    j = 2
    while j < nqb:
        w = min(2, nqb - j)
        rest.append((2 * j - 2, "k", j, w))
        j += w
    for g in range(len(vgroups)):
        s0 = vgroups[g][0]
        if s0 < 8:
            continue                              # covered by warmup DMAs
        rest.append((max(0, s0 // 2 - 1), "v", s0 // 4, min(2, nqb - s0 // 4)))
    for _, kind, j, w in sorted(rest, key=lambda r: r[0]):
        dma_x(xk_sb if kind == "k" else xv_sb, xk if kind == "k" else xv,
              j, width=w)
    j = 1
    while j < nqb:
        w = min(2, nqb - j)
        dma_x(xq_sb, xq, j, width=w)
        j += w

    # --- projection emitters ---
    def proj_block(x_sb, w_sb, bias, dstT, j, evac_act=False):
        qsl = slice(j * QB, (j + 1) * QB)
        acc = scp.tile([128, SCW], F32, tag="sc", name=f"prj{j}")[:, 0:QB]
        for c in range(NCH):
            nc.tensor.matmul(
                acc, lhsT=w_sb[:, c, :], rhs=x_sb[c][:, qsl],
                start=(c == 0), stop=(c == NCH - 1),
            )
        if evac_act:  # ACT is idle during warmup; parallels the DVE evac
            nc.scalar.activation(
                dstT[:, qsl], acc, mybir.ActivationFunctionType.Identity,
                bias=bias[:, 0:1],
            )
        elif bias is None:
            nc.vector.tensor_copy(out=dstT[:, qsl], in_=acc)
        else:
            nc.vector.tensor_scalar(dstT[:, qsl], acc, bias[:, 0:1], None, ADD)

    def vproj_group(g):
        sts = vgroups[g]
        acc = scp.tile([128, SCW], F32, tag="sc", name=f"vprj{g}")
        for i, st in enumerate(sts):
            a = acc[:, i * 128 : (i + 1) * 128]
            for c in range(NCH):
                nc.tensor.matmul(
                    a, lhsT=xv_sb[c][:, st * 128 : (st + 1) * 128],
                    rhs=wv_sb[:, c, :],
                    start=(c == 0), stop=(c == NCH - 1),
                )
        av_view = acc.rearrange("p (i m) -> p i m", m=128)[:, 0 : len(sts), :]
        for h in range(HPC):
            dst = vh[h].rearrange("p (j c) -> p j c", c=65)[
                :, sts[0] : sts[0] + len(sts), 0:64
            ]
            nc.vector.tensor_copy(out=dst, in_=av_view[:, :, h * 64 : (h + 1) * 64])

    # --- injection schedule: projections emitted inside qb0's ktg loop ---
    inj = defaultdict(list)
    for j in range(1, nqb):                       # K block j needed at ktg 2j
        inj[(0, max(0, min(2 * j - 2, nktg - 1)))].append(("K", j))
    for g in range(1, len(vgroups)):              # V grp needed at ktg s0/2
        s0 = vgroups[g][0]
        inj[(0, max(0, min(s0 // 2 - 1, nktg - 1)))].append(("V", g))
    for qb in range(nqb - 1):                     # Q block qb+1 inside qb
        inj[(qb, max(0, nktg - 4))].append(("Q", qb + 1))

    # upfront: K block0, Q block0, V group0
    proj_block(xk_sb, wk_sb, bk_sb, khT, 0)
    proj_block(xq_sb, wq_sb, bq_sb, qhT, 0, evac_act=True)
    vproj_group(0)

    def emit_av(av_t, ktg, ex):
        # ex[part] is [128, 1024] = [h0 cols 0:512 | h1 cols 512:1024] for
        # key tile kt_i = 2*ktg + part
        for part in range(KPG):
            kt_i = KPG * ktg + part
            for h in range(HPC):
                nc.tensor.matmul(
                    av_t[h][0:65, :],
                    lhsT=vh[h][:, kt_i * 65 : kt_i * 65 + 65],
                    rhs=ex[part][:, h * QB : (h + 1) * QB],
                    start=(ktg == 0 and part == 0),
                    stop=(ktg == nktg - 1 and part == KPG - 1),
                )

    # --- normalize + output projection (deferred into the next qblock) ---
    def norm_a(nstate):
        """Right after the qblock's last AV matmul: evacuate the av PSUM banks
        (rows 0-64 incl. the denominator row) to bf16 (h0 on DVE, h1 on ACT),
        move denominators to a q-on-partitions layout (tiny K=1 matmuls),
        one wide reciprocal."""
        av_t, st8 = nstate["av"], nstate["st8"]
        av_sb = []
        for h in range(HPC):
            a = sb.tile([65, QB], BF16, tag=f"avs{h}", bufs=2, name=f"avs{h}")
            if h == 0:
                nc.vector.tensor_copy(out=a, in_=av_t[h][0:65, :])
            else:
                nc.scalar.copy(a, av_t[h][0:65, :])
            av_sb.append(a)
        denT = scp.tile([128, SCW], F32, tag="sc", name="denT")
        for h in range(HPC):
            for st in range(NSUB):
                j = h * NSUB + st
                o = st * 128
                nc.tensor.matmul(
                    denT[:, j : j + 1],
                    lhsT=av_sb[h][64:65, o : o + 128],
                    rhs=ones_bf[64:65, 0:1],
                    start=True, stop=True,
                )
        rt_sb = sb.tile([128, 2 * NSUB], F32, tag="rt", bufs=2, name="rt_sb")
        nc.vector.reciprocal(
            out=rt_sb, in_=denT[:, 0 : 2 * NSUB]
        )
        st8["rt"] = rt_sb
        st8["av_sb"] = av_sb

    def norm_b(nstate):
        """One out-projection subtile: per-head K=64 matmuls into op halves,
        per-partition 1/den scale at evacuation, per-head bf16 partials DMA'd
        out (the h0+h1 sum happens in the host gather with the core sum)."""
        st = nstate["sub"]
        nstate["sub"] += 1
        qb, st8 = nstate["qb"], nstate["st8"]
        rt_sb, av_sb = st8["rt"], st8["av_sb"]
        ssl = slice(st * 128, (st + 1) * 128)
        op = scp.tile([128, SCW], F32, tag="sc", name="op")
        base = qb * QB + st * 128
        last_qb = nstate["last"]
        for h in range(HPC):
            nc.tensor.matmul(
                op[:, h * QB : (h + 1) * QB],
                lhsT=av_sb[h][0:64, ssl],
                rhs=(wo0_sb if h == 0 else wo1_sb),
                start=True, stop=True,
            )
            t = sb.tile([128, QB], BF16, tag="ot", bufs=4, name=f"ot{h}")
            rt_col = rt_sb[:, h * NSUB + st : h * NSUB + st + 1]
            if h == 0:
                nc.vector.tensor_scalar(
                    t, op[:, h * QB : (h + 1) * QB], rt_col, None, MULT,
                )
            else:
                nc.scalar.activation(
                    t, op[:, h * QB : (h + 1) * QB],
                    mybir.ActivationFunctionType.Copy, scale=rt_col,
                )
            # during the run the Sync queue issues these; on the final qblock
            # ACT's queue is idle so split across both to shorten the tail
            dq = nc.scalar if (last_qb and h == 1) else nc.sync
            dq.dma_start(out_ap[h * seq + base : h * seq + base + 128, :], t)

    # --- attention ---
    nstate = None
    for qb in range(nqb):
        qsl = slice(qb * QB, (qb + 1) * QB)
        av_t = [
            avp.tile([128, QB], F32, tag="av", name=f"av{h}") for h in range(HPC)
        ]
        prev = None
        for ktg in range(nktg):
            for kind, arg in inj.get((qb, ktg), ()):
                if kind == "K":
                    proj_block(xk_sb, wk_sb, bk_sb, khT, arg)
                elif kind == "Q":
                    proj_block(xq_sb, wq_sb, bq_sb, qhT, arg)
                else:
                    vproj_group(arg)
            # scores: per key tile, BOTH heads into one psum tile
            # ([h0 cols 0:512 | h1 cols 512:1024]) so the row-tiled K=64 pair
            # shares its psum-free dependency and issues adjacently -> the
            # two matmuls co-execute in the PE array.
            sc_t = [
                scp.tile([128, SCW], F32, tag="sc", name=f"sc{part}")
                for part in range(KPG)
            ]
            part_order = [1, 0] if ktg == nktg - 1 else [0, 1]
            for part in part_order:
                kt_i = KPG * ktg + part
                ksl = slice(kt_i * KT, (kt_i + 1) * KT)
                for h in range(HPC):
                    hp = slice(h * 64, (h + 1) * 64)
                    nc.tensor.matmul(
                        sc_t[part][:, h * QB : (h + 1) * QB],
                        lhsT=khT[hp, ksl], rhs=qhT[hp, qsl],
                        start=True, stop=True,
                    )
            # exp: ACT or DVE (Schraudolph) per schedule
            ex = []
            for part in range(KPG):
                if part == 1 and ktg >= dve_part_min:
                    e = sb.tile([128, SCW], I16, tag="exi", bufs=4, name="exi")
                    nc.vector.tensor_scalar(
                        e[:, :], sc_t[part], EXP_C1, EXP_C2, MULT, ADD
                    )
                    ex.append(e[:, :].bitcast(BF16))
                else:
                    e = sb.tile([128, SCW], BF16, tag="exb", bufs=5, name="exb")
                    nc.scalar.activation(e, sc_t[part], EXP)
                    ex.append(e[:, :])
            # deferred out-projection of the previous qblock: one subtile per
            # even ktg slot (DVE-exp slots are odd, so the evacuation work
            # never queues behind a DVE exp tile)
            if (
                nstate is not None
                and ktg >= 2
                and (ktg - 2) % 4 == 0
                and nstate["sub"] < NSUB
            ):
                norm_b(nstate)
            if prev is not None:
                emit_av(av_t, *prev)
            prev = (ktg, ex)
        emit_av(av_t, *prev)
        if nstate is not None:  # flush any remaining subtiles (small-seq case)
            while nstate["sub"] < NSUB:
                norm_b(nstate)
        # normalization prep right after this qblock's last AV matmul
        nstate = {"av": av_t, "qb": qb, "st8": {}, "sub": 0, "last": qb == nqb - 1}
        norm_a(nstate)
    while nstate["sub"] < NSUB:
        norm_b(nstate)

    avp.release()
    scp.release()
    sb.release()
    const.release()


def build_bass(seq=S, dve_part_min=DVE_PART_MIN):
    nc = bacc.Bacc(
        "TRN2",
        debug=False,
        enable_asserts=False,
        target_bir_lowering=False,
    )
    ins = {}
    shapes = {
        "qt": (D, seq), "kt": (D, seq), "vt": (D, seq),
        "wq": (128, NCH * HD), "wk": (128, NCH * HD), "wv": (128, NCH * HD),
        "wo": (128, D),
        "bq": (HD, 1), "bk": (HD, 1),
    }
    bf16_names = {"qt", "kt", "vt", "wq", "wk", "wv", "wo"}
    for name, shape in shapes.items():
        dt = BF16 if name in bf16_names else F32
        ins[name] = nc.dram_tensor(name, list(shape), dt, kind="ExternalInput").ap()
    # per-head unnormalized... normalized partials: rows [0:seq] = head0,
    # [seq:2*seq] = head1; summed with the cross-core partials in the gather
    out = nc.dram_tensor("out", [HPC * seq, D], BF16, kind="ExternalOutput").ap()
    with tile.TileContext(nc) as tc:
        mha_tile_kernel(tc, out, ins, seq=seq, dve_part_min=dve_part_min)
    nc.compile()
    return nc


def shard_inputs(q, k, v, Wq, bq, Wk, bk, Wv, bv, Wo, bo, seq=S):
    """Host-side shard prep. Returns (in_maps, const_vec)."""
    scale = 1.0 / np.sqrt(np.float32(DK))
    q, k, v = (np.asarray(x, np.float32) for x in (q, k, v))
    Wq, bq, Wk, bk, Wv, bv, Wo, bo = (
        np.asarray(x, np.float32) for x in (Wq, bq, Wk, bk, Wv, bv, Wo, bo)
    )
    bf = lambda x: np.ascontiguousarray(x).astype(NPBF16)
    # device weight layout [p, c*m]: w_dev[p, c, m] = w.T[c*128 + p, m]
    wperm = lambda w: (
        w.T.reshape(NCH, 128, HD).transpose(1, 0, 2).reshape(128, NCH * HD)
    )
    in_maps = []
    for c in range(N_CORES):
        b = c // 4
        rows = slice(128 * (c % 4), 128 * (c % 4) + 128)
        in_maps.append({
            "qt": bf(q[b].T),
            "kt": bf(k[b].T),
            "vt": bf(v[b].T),
            "wq": bf(wperm(Wq[rows, :] * scale)),
            "wk": bf(wperm(Wk[rows, :])),
            "wv": bf(wperm(Wv[rows, :])),
            "wo": bf(Wo[:, rows].T),
            "bq": np.ascontiguousarray((bq[rows] * scale).reshape(HD, 1)),
            "bk": np.ascontiguousarray(bk[rows].reshape(HD, 1)),
        })
    const_vec = (bv @ Wo.T + bo).astype(np.float32)
    return in_maps, const_vec


_NC_CACHE = {}


def run(inputs, seq=S, trace=False, trace_kwargs=None):
    if seq not in _NC_CACHE:
        _NC_CACHE[seq] = build_bass(seq=seq)
    nc = _NC_CACHE[seq]
    in_maps, const_vec = shard_inputs(**inputs, seq=seq)
    res = run_bass_kernel_spmd(
        nc,
        in_maps,
        core_ids=list(range(N_CORES)),
        trace=trace,
        **(trace_kwargs or {}),
    )
    out = np.zeros((B, seq, D), dtype=np.float32)
    for c in range(N_CORES):
        r = np.asarray(res.results[c]["out"], dtype=np.float32)
        out[c // 4] += r[0:seq] + r[seq : 2 * seq]
    out += const_vec[None, None, :]
    return out, res


def kernel(**inputs):
    out, _ = run(inputs)
    return out



# revision 57
# speedup vs baseline: 1.0135x; 1.0135x over previous
"""Multi-head attention (B=2, S=4096, D=512, H=8) on 8 Trainium2 NeuronCores.

Sharding: batch x head-pair parallelism. Core c handles batch b = c // 4 and
heads {2*(c%4), 2*(c%4)+1} (128 contiguous rows of the QKV projection
weights, Megatron column-parallel; Wo row-parallel with the partial-sum
reduction done on the host at gather time).

Per-core device program (matmul operands bf16, accumulation fp32 PSUM).
The PE array is the critical engine (~85% busy); the structure keeps its
matmul stream dense while ACT and DVE split the exp/normalize load:
  - Input DMA on both HWDGE queues (Sync + Activation) in parallel,
    need-ordered; the Activation queue carries only the small warmup set
    so its FIFO is clear before the first exp (queued DMA issues block on
    DMA-ring credits). K-block0/Q-block0/V-group0 projected up front;
    remaining blocks injected just-in-time into qb0's ktg loop.
  - Scores per key tile: ONE [128(k), 1024] f32 PSUM tile holds BOTH
    heads ([h0 512 | h1 512]), so the row-tiled K=64 pair (h0 partitions
    0-63, h1 64-127) shares its psum-free dependency, issues adjacently,
    and co-executes in the PE array (2x score throughput).
  - exp: part 0 of each ktg on ACT (exact exp, PSUM->SBUF bf16), part 1
    on DVE via a Schraudolph bit-hack (int16(x*128/ln2 + 16251) bitcast
    bf16, ~3% max rel err). Splitting each ktg across both engines halves
    exp latency per ktg, so the lag-1 AV drain at qblock boundaries is
    short, and the engines stay load-balanced.
  - AV: vh tiles [128, 65] (65th col = ones -> softmax denominator) as
    stationary; accumulated over all 32 key tiles directly in PSUM
    ([65, 512] per head, held across the qblock), one ktg behind scores.
  - Normalize (deferred into the next qblock): av+den evacuated in one
    bf16 cast per head (h0 on DVE, h1 on ACT), denominators moved to a
    q-on-partitions layout with tiny K=1 matmuls, ONE [128, 8] exact
    reciprocal, per-head output projection, 1/den applied per-partition
    at evacuation (h0 DVE tensor_scalar, h1 ACT activation-scale), and
    per-head bf16 partials DMA'd out separately (rows [0:S] = head0,
    [S:2S] = head1) -- the head sum rides the host gather.

Host gathers: out[b] = sum over 4 cores of (head0 + head1 partials)
+ bv @ Wo.T + bo.
"""

from collections import defaultdict

import ml_dtypes
import numpy as np

import concourse.mybir as mybir
import concourse.tile as tile
from concourse import bacc
from concourse.bass_utils import run_bass_kernel_spmd

F32 = mybir.dt.float32
BF16 = mybir.dt.bfloat16
I16 = mybir.dt.int16
EXP = mybir.ActivationFunctionType.Exp
ADD = mybir.AluOpType.add
MULT = mybir.AluOpType.mult
NPBF16 = ml_dtypes.bfloat16

B, S, D, H = 2, 4096, 512, 8
DK = D // H          # 64
HPC = 2              # heads per core
HD = HPC * DK        # 128 head-dims per core
N_CORES = 8
QB = 512             # query block (matmul free dim)
KT = 128             # key tile (partition dim)
NCH = D // 128       # 4 contraction chunks for the projections
KPG = 2              # key tiles per score/exp group
SCW = KPG * QB       # score tile width (1024)
NSUB = QB // 128     # out-projection subtiles per qblock (4)

# Schraudolph bf16 exp: exp(s) ~= bitcast_bf16(int16(s*C1 + C2)); C1 = 2^7/ln2,
# C2 = 127*128 - sigma with sigma tuned for truncating f32->int16 conversion.
EXP_C1 = 184.6649652337873
EXP_C2 = 16251.0
# exp engine assignment: part 0 of each ktg runs on ACT, part 1 on DVE for
# ktg >= DVE_PART_MIN (the first ktgs go all-ACT to balance DVE's fixed
# norm/copy load). Splitting each ktg across both engines halves the exp
# latency per ktg, so the lag-1 AV drain at the qblock boundary is short.
DVE_PART_MIN = 2


def mha_tile_kernel(tc, out_ap, ins, seq=S, dve_part_min=DVE_PART_MIN):
    nc = tc.nc
    nqb, nkt = seq // QB, seq // KT
    nktg = nkt // KPG
    nst = seq // 128                      # 128-wide s-subtiles for V
    vgroups = [range(0, min(4, nst)), range(4, min(8, nst))] + [
        range(g, min(g + 8, nst)) for g in range(8, nst, 8)
    ]
    vgroups = [g for g in vgroups if len(g) > 0]

    xq, xk, xv = ins["qt"], ins["kt"], ins["vt"]
    const = tc.alloc_tile_pool(name="const", bufs=1)
    sb = tc.alloc_tile_pool(name="sb", bufs=2)
    scp = tc.alloc_tile_pool(name="scp", bufs=3, space="PSUM")
    avp = tc.alloc_tile_pool(name="avp", bufs=2, space="PSUM")

    # --- weights / constants ---
    wq_sb = const.tile([128, NCH, 128], BF16, tag="wq", name="wq_sb")
    wk_sb = const.tile([128, NCH, 128], BF16, tag="wk", name="wk_sb")
    wv_sb = const.tile([128, NCH, 128], BF16, tag="wv", name="wv_sb")
    wo0_sb = const.tile([64, QB], BF16, tag="wo0", name="wo0_sb")
    wo1_sb = const.tile([64, QB], BF16, tag="wo1", name="wo1_sb")
    bq_sb = const.tile([128, 1], F32, tag="bq", name="bq_sb")
    bk_sb = const.tile([128, 1], F32, tag="bk", name="bk_sb")
    ones_sb = const.tile([128, 64], F32, tag="ones", name="ones_sb")
    nc.vector.memset(ones_sb, 1.0)
    ones_bf = const.tile([128, 1], BF16, tag="onesb", name="ones_bf")
    nc.vector.memset(ones_bf, 1.0)

    # --- persistent activations ---
    qhT = const.tile([128, seq], BF16, tag="qhT", name="qhT")
    khT = const.tile([128, seq], BF16, tag="khT", name="khT")
    vh = [
        const.tile([128, nkt * 65], BF16, tag=f"vh{h}", name=f"vh{h}")
        for h in range(HPC)
    ]
    for h in range(HPC):
        ones_col = vh[h].rearrange("p (j c) -> p j c", c=65)[:, :, 64]
        nc.vector.tensor_copy(out=ones_col, in_=ones_sb[:, 0:nkt])

    # --- raw inputs in SBUF; DMA'd in [128, 512] slices in consumption order
    xk_sb = [const.tile([128, seq], BF16, tag=f"xk{c}", name=f"xk{c}") for c in range(NCH)]
    xq_sb = [const.tile([128, seq], BF16, tag=f"xq{c}", name=f"xq{c}") for c in range(NCH)]
    xv_sb = [const.tile([128, seq], BF16, tag=f"xv{c}", name=f"xv{c}") for c in range(NCH)]

    def dma_x(dst_tiles, src, j, eng=None, width=1):
        sl = slice(j * QB, (j + width) * QB)
        for c in range(NCH):
            (eng or nc.sync).dma_start(
                dst_tiles[c][:, sl], src[c * 128 : (c + 1) * 128, sl]
            )

    # DMAs issue serially per queue (~640ns each after a ~6.6us preamble);
    # both the Sync and Activation queues are HWDGE-capable, so the warmup
    # transfers (K0 on scalar, Q0 on sync) issue in parallel. Weights are
    # host-permuted to [p, c, m] so their DMA is contiguous.
    # Two HWDGE queues (Sync + Activation) issue in parallel. The Scalar
    # queue gets ONLY the small warmup set (its FIFO must be clear before
    # the first exp ACTIVATE -- queued DMA issues block on DMA-ring credits
    # for tens of us). Everything else goes need-ordered on Sync.
    nc.scalar.dma_start(wk_sb, ins["wk"].rearrange("p (c m) -> p c m", m=128))
    dma_x(xk_sb, xk, 0, eng=nc.scalar)
    nc.scalar.dma_start(bk_sb, ins["bk"])
    nc.scalar.dma_start(wv_sb, ins["wv"].rearrange("p (c m) -> p c m", m=128))
    dma_x(xv_sb, xv, 0, eng=nc.scalar)            # v subtiles 0-3 @ qb0 ktg1
    nc.sync.dma_start(wq_sb, ins["wq"].rearrange("p (c m) -> p c m", m=128))
    dma_x(xq_sb, xq, 0, eng=nc.sync)
    nc.sync.dma_start(bq_sb, ins["bq"])
    if nqb > 1:
        dma_x(xk_sb, xk, 1, eng=nc.sync)          # K1-proj @ qb0 ktg0
        dma_x(xv_sb, xv, 1, eng=nc.sync)          # v subtiles 4-7 @ qb0 ktg1
    nc.sync.dma_start(wo0_sb, ins["wo"][0:64, :])  # first norm_b in qb1
    nc.sync.dma_start(wo1_sb, ins["wo"][64:128, :])
    # remaining blocks on Sync by first-use slot (K j @ ktg 2j-2, V grp g
    # @ 4g-1); 1024-wide slices to halve the issue count
    rest = []
    j = 2
    while j < nqb:
        w = min(2, nqb - j)
        rest.append((2 * j - 2, "k", j, w))
        j += w
    for g in range(len(vgroups)):
        s0 = vgroups[g][0]
        if s0 < 8:
            continue                              # covered by warmup DMAs
        rest.append((max(0, s0 // 2 - 1), "v", s0 // 4, min(2, nqb - s0 // 4)))
    for _, kind, j, w in sorted(rest, key=lambda r: r[0]):
        dma_x(xk_sb if kind == "k" else xv_sb, xk if kind == "k" else xv,
              j, width=w)
    j = 1
    while j < nqb:
        w = min(2, nqb - j)
        dma_x(xq_sb, xq, j, width=w)
        j += w

    # --- projection emitters ---
    def proj_block(x_sb, w_sb, bias, dstT, j, evac_act=False):
        qsl = slice(j * QB, (j + 1) * QB)
        acc = scp.tile([128, SCW], F32, tag="sc", name=f"prj{j}")[:, 0:QB]
        for c in range(NCH):
            nc.tensor.matmul(
                acc, lhsT=w_sb[:, c, :], rhs=x_sb[c][:, qsl],
                start=(c == 0), stop=(c == NCH - 1),
            )
        if evac_act:  # ACT is idle during warmup; parallels the DVE evac
            nc.scalar.activation(
                dstT[:, qsl], acc, mybir.ActivationFunctionType.Identity,
                bias=bias[:, 0:1],
            )
        elif bias is None:
            nc.vector.tensor_copy(out=dstT[:, qsl], in_=acc)
        else:
            nc.vector.tensor_scalar(dstT[:, qsl], acc, bias[:, 0:1], None, ADD)

    def vproj_group(g):
        sts = vgroups[g]
        acc = scp.tile([128, SCW], F32, tag="sc", name=f"vprj{g}")
        for i, st in enumerate(sts):
            a = acc[:, i * 128 : (i + 1) * 128]
            for c in range(NCH):
                nc.tensor.matmul(
                    a, lhsT=xv_sb[c][:, st * 128 : (st + 1) * 128],
                    rhs=wv_sb[:, c, :],
                    start=(c == 0), stop=(c == NCH - 1),
                )
        av_view = acc.rearrange("p (i m) -> p i m", m=128)[:, 0 : len(sts), :]
        for h in range(HPC):
            dst = vh[h].rearrange("p (j c) -> p j c", c=65)[
                :, sts[0] : sts[0] + len(sts), 0:64
            ]
            nc.vector.tensor_copy(out=dst, in_=av_view[:, :, h * 64 : (h + 1) * 64])

    # --- injection schedule: projections emitted inside qb0's ktg loop ---
    inj = defaultdict(list)
    for j in range(1, nqb):                       # K block j needed at ktg 2j
        inj[(0, max(0, min(2 * j - 2, nktg - 1)))].append(("K", j))
    for g in range(1, len(vgroups)):              # V grp needed at ktg s0/2
        s0 = vgroups[g][0]
        inj[(0, max(0, min(s0 // 2 - 1, nktg - 1)))].append(("V", g))
    for qb in range(nqb - 1):                     # Q block qb+1 inside qb
        inj[(qb, max(0, nktg - 4))].append(("Q", qb + 1))

    # upfront: K block0, Q block0, V group0
    proj_block(xk_sb, wk_sb, bk_sb, khT, 0)
    proj_block(xq_sb, wq_sb, bq_sb, qhT, 0, evac_act=True)
    vproj_group(0)

    def emit_av(av_t, ktg, ex):
        # ex[part] is [128, 1024] = [h0 cols 0:512 | h1 cols 512:1024] for
        # key tile kt_i = 2*ktg + part
        for part in range(KPG):
            kt_i = KPG * ktg + part
            for h in range(HPC):
                nc.tensor.matmul(
                    av_t[h][0:65, :],
                    lhsT=vh[h][:, kt_i * 65 : kt_i * 65 + 65],
                    rhs=ex[part][:, h * QB : (h + 1) * QB],
                    start=(ktg == 0 and part == 0),
                    stop=(ktg == nktg - 1 and part == KPG - 1),
                )

    # --- normalize + output projection (deferred into the next qblock) ---
    def norm_a(nstate):
        """Right after the qblock's last AV matmul: evacuate the av PSUM banks
        (rows 0-64 incl. the denominator row) to bf16 (h0 on DVE, h1 on ACT),
        move denominators to a q-on-partitions layout (tiny K=1 matmuls),
        one wide reciprocal."""
        av_t, st8 = nstate["av"], nstate["st8"]
        av_sb = []
        for h in range(HPC):
            a = sb.tile([65, QB], BF16, tag=f"avs{h}", bufs=2, name=f"avs{h}")
            if h == 0:
                nc.vector.tensor_copy(out=a, in_=av_t[h][0:65, :])
            else:
                nc.scalar.copy(a, av_t[h][0:65, :])
            av_sb.append(a)
        denT = scp.tile([128, SCW], F32, tag="sc", name="denT")
        for h in range(HPC):
            for st in range(NSUB):
                j = h * NSUB + st
                o = st * 128
                nc.tensor.matmul(
                    denT[:, j : j + 1],
                    lhsT=av_sb[h][64:65, o : o + 128],
                    rhs=ones_bf[64:65, 0:1],
                    start=True, stop=True,
                )
        rt_sb = sb.tile([128, 2 * NSUB], F32, tag="rt", bufs=2, name="rt_sb")
        nc.vector.reciprocal(
            out=rt_sb, in_=denT[:, 0 : 2 * NSUB]
        )
        st8["rt"] = rt_sb
        st8["av_sb"] = av_sb

    def norm_b(nstate):
        """One out-projection subtile: per-head K=64 matmuls into op halves,
        per-partition 1/den scale at evacuation, per-head bf16 partials DMA'd
        out (the h0+h1 sum happens in the host gather with the core sum)."""
        st = nstate["sub"]
        nstate["sub"] += 1
        qb, st8 = nstate["qb"], nstate["st8"]
        rt_sb, av_sb = st8["rt"], st8["av_sb"]
        ssl = slice(st * 128, (st + 1) * 128)
        op = scp.tile([128, SCW], F32, tag="sc", name="op")
        base = qb * QB + st * 128
        last_qb = nstate["last"]
        for h in range(HPC):
            nc.tensor.matmul(
                op[:, h * QB : (h + 1) * QB],
                lhsT=av_sb[h][0:64, ssl],
                rhs=(wo0_sb if h == 0 else wo1_sb),
                start=True, stop=True,
            )
            t = sb.tile([128, QB], BF16, tag="ot", bufs=4, name=f"ot{h}")
            rt_col = rt_sb[:, h * NSUB + st : h * NSUB + st + 1]
            if h == 0:
                nc.vector.tensor_scalar(
                    t, op[:, h * QB : (h + 1) * QB], rt_col, None, MULT,
                )
            else:
                nc.scalar.activation(
                    t, op[:, h * QB : (h + 1) * QB],
                    mybir.ActivationFunctionType.Copy, scale=rt_col,
                )
            # during the run the Sync queue issues these; on the final qblock
            # ACT's queue is idle so split across both to shorten the tail
            dq = nc.scalar if (last_qb and h == 1) else nc.sync
            dq.dma_start(out_ap[h * seq + base : h * seq + base + 128, :], t)

    # --- attention ---
    nstate = None
    for qb in range(nqb):
        qsl = slice(qb * QB, (qb + 1) * QB)
        av_t = [
            avp.tile([128, QB], F32, tag="av", name=f"av{h}") for h in range(HPC)
        ]
        prev = None
        for ktg in range(nktg):
            for kind, arg in inj.get((qb, ktg), ()):
                if kind == "K":
                    proj_block(xk_sb, wk_sb, bk_sb, khT, arg)
                elif kind == "Q":
                    proj_block(xq_sb, wq_sb, bq_sb, qhT, arg)
                else:
                    vproj_group(arg)
            # scores: per key tile, BOTH heads into one psum tile
            # ([h0 cols 0:512 | h1 cols 512:1024]) so the row-tiled K=64 pair
            # shares its psum-free dependency and issues adjacently -> the
            # two matmuls co-execute in the PE array.
            sc_t = [
                scp.tile([128, SCW], F32, tag="sc", name=f"sc{part}")
                for part in range(KPG)
            ]
            part_order = [1, 0] if ktg == nktg - 1 else [0, 1]
            for part in part_order:
                kt_i = KPG * ktg + part
                ksl = slice(kt_i * KT, (kt_i + 1) * KT)
                for h in range(HPC):
                    hp = slice(h * 64, (h + 1) * 64)
                    nc.tensor.matmul(
                        sc_t[part][:, h * QB : (h + 1) * QB],
                        lhsT=khT[hp, ksl], rhs=qhT[hp, qsl],
                        start=True, stop=True,
                    )
            # exp: ACT or DVE (Schraudolph) per schedule
            ex = []
            for part in range(KPG):
                if part == 1 and ktg >= dve_part_min:
                    e = sb.tile([128, SCW], I16, tag="exi", bufs=4, name="exi")
                    nc.vector.tensor_scalar(
                        e[:, :], sc_t[part], EXP_C1, EXP_C2, MULT, ADD
                    )
                    ex.append(e[:, :].bitcast(BF16))
                else:
                    e = sb.tile([128, SCW], BF16, tag="exb", bufs=5, name="exb")
                    nc.scalar.activation(e, sc_t[part], EXP)
                    ex.append(e[:, :])
            # deferred out-projection of the previous qblock: one subtile per
            # even ktg slot (DVE-exp slots are odd, so the evacuation work
            # never queues behind a DVE exp tile)
            if (
                nstate is not None
                and ktg >= 2
                and (ktg - 2) % 4 == 0
                and nstate["sub"] < NSUB
            ):
                norm_b(nstate)
            if prev is not None:
                emit_av(av_t, *prev)
            prev = (ktg, ex)
        emit_av(av_t, *prev)
        if nstate is not None:  # flush any remaining subtiles (small-seq case)
            while nstate["sub"] < NSUB:
                norm_b(nstate)
        # normalization prep right after this qblock's last AV matmul
        nstate = {"av": av_t, "qb": qb, "st8": {}, "sub": 0, "last": qb == nqb - 1}
        norm_a(nstate)
    while nstate["sub"] < NSUB:
        norm_b(nstate)

    avp.release()
    scp.release()
    sb.release()
    const.release()


def build_bass(seq=S, dve_part_min=DVE_PART_MIN):
    nc = bacc.Bacc(
        "TRN2",
        debug=False,
        enable_asserts=False,
        target_bir_lowering=False,
    )
    ins = {}
    shapes = {
        "qt": (D, seq), "kt": (D, seq), "vt": (D, seq),
        "wq": (128, NCH * HD), "wk": (128, NCH * HD), "wv": (128, NCH * HD),
        "wo": (128, D),
        "bq": (HD, 1), "bk": (HD, 1),
    }
    bf16_names = {"qt", "kt", "vt", "wq", "wk", "wv", "wo"}
    for name, shape in shapes.items():
        dt = BF16 if name in bf16_names else F32
        ins[name] = nc.dram_tensor(name, list(shape), dt, kind="ExternalInput").ap()
    # per-head unnormalized... normalized partials: rows [0:seq] = head0,
    # [seq:2*seq] = head1; summed with the cross-core partials in the gather
    out = nc.dram_tensor("out", [HPC * seq, D], BF16, kind="ExternalOutput").ap()
    with tile.TileContext(nc) as tc:
        mha_tile_kernel(tc, out, ins, seq=seq, dve_part_min=dve_part_min)
    nc.compile()
    return nc


def shard_inputs(q, k, v, Wq, bq, Wk, bk, Wv, bv, Wo, bo, seq=S):
    """Host-side shard prep. Returns (in_maps, const_vec)."""
    scale = 1.0 / np.sqrt(np.float32(DK))
    q, k, v = (np.asarray(x, np.float32) for x in (q, k, v))
    Wq, bq, Wk, bk, Wv, bv, Wo, bo = (
        np.asarray(x, np.float32) for x in (Wq, bq, Wk, bk, Wv, bv, Wo, bo)
    )
    bf = lambda x: np.ascontiguousarray(x).astype(NPBF16)
    # device weight layout [p, c*m]: w_dev[p, c, m] = w.T[c*128 + p, m]
    wperm = lambda w: (
        w.T.reshape(NCH, 128, HD).transpose(1, 0, 2).reshape(128, NCH * HD)
    )
    in_maps = []
    for c in range(N_CORES):
        b = c // 4
        rows = slice(128 * (c % 4), 128 * (c % 4) + 128)
        in_maps.append({
            "qt": bf(q[b].T),
            "kt": bf(k[b].T),
            "vt": bf(v[b].T),
            "wq": bf(wperm(Wq[rows, :] * scale)),
            "wk": bf(wperm(Wk[rows, :])),
            "wv": bf(wperm(Wv[rows, :])),
            "wo": bf(Wo[:, rows].T),
            "bq": np.ascontiguousarray((bq[rows] * scale).reshape(HD, 1)),
            "bk": np.ascontiguousarray(bk[rows].reshape(HD, 1)),
        })
    const_vec = (bv @ Wo.T + bo).astype(np.float32)
    return in_maps, const_vec


_NC_CACHE = {}


def run(inputs, seq=S, trace=False, trace_kwargs=None):
    if seq not in _NC_CACHE:
        _NC_CACHE[seq] = build_bass(seq=seq)
    nc = _NC_CACHE[seq]
    in_maps, const_vec = shard_inputs(**inputs, seq=seq)
    res = run_bass_kernel_spmd(
        nc,
        in_maps,
        core_ids=list(range(N_CORES)),
        trace=trace,
        **(trace_kwargs or {}),
    )
    out = np.zeros((B, seq, D), dtype=np.float32)
    for c in range(N_CORES):
        r = np.asarray(res.results[c]["out"], dtype=np.float32)
        out[c // 4] += r[0:seq] + r[seq : 2 * seq]
    out += const_vec[None, None, :]
    return out, res


def kernel(**inputs):
    out, _ = run(inputs)
    return out

